# revision 1
# baseline (speedup 1.0000x reference)
"""GCN (8-layer, 16 GCNConv) on 8 TRN2 NeuronCores.

Strategy (v2 — software-pipelined across convs):
- dst-partition nodes across 8 cores (6250 each); weights replicated.
- norm separability: norm[e] = dis[src]*dis[dst], so each conv is
    g = dis * (h @ W)         (node-major, per-core slice, bf16)
    AllGather g (split into two half-collectives a/b)
    agg[f,d] = sum_e g_fm[src[e]] onehot[e,d]   via PE matmuls over
               128-edge chunks (msgs gathered edge-major by SWDGE dma_gather)
    h' = relu?(dis * agg + b)  (feature-major)
- self-loop folded in as the START matmul of each block's a-phase PSUM
  accumulation chain (g_nm block x ident), so agg is written once per
  phase: a-phase via scalar-engine activation copy (PSUM->SBUF), b-phase
  via one DVE add.
- per-block epilogue fused into the b-phase loop, immediately followed by
  the NEXT conv's h@W matmul + g_nm scale for that block, so the next
  conv's half-AllGathers are issued while the current conv's scatter is
  still draining (cross-conv software pipeline).
- h is stored bf16 (h16); agg accumulates f32; matmuls run bf16.
- edges are host-sorted by (stream, dst block); per-block chunk counts are
  shared across cores (max), pad slots gather row 0 with onehot id -1.
- int16 gather indices: the a/b split keeps indices < 25600.
- final mean-pool via matmul with host-built pooling matrix + AllReduce.
"""
import numpy as np
import concourse.bass as bass
import concourse.mybir as mybir
import concourse.bacc as bacc
import concourse.tile as tile
from concourse.bass_utils import run_bass_kernel_spmd

import os
N = 50000
E = 600000
D = 128
L = 8
NCONV = int(os.environ.get("GCN_NCONV", 2 * L))
SKIP_COLL = os.environ.get("GCN_SKIP_COLL", "") == "1"
SKIP_GDMA = os.environ.get("GCN_SKIP_GDMA", "") == "1"
SKIP_STREAMS = os.environ.get("GCN_SKIP_STREAMS", "") == "1"
SKIP_GATHER = os.environ.get("GCN_SKIP_GATHER", "") == "1"
DUMP_H = os.environ.get("GCN_DUMP_H", "") == "1"
H16 = os.environ.get("GCN_H16", "1") == "1"
AG_NODEP = os.environ.get("GCN_AG_NODEP", "") == "1"
TAILFOLD = os.environ.get("GCN_TAILFOLD", "1") == "1"
C = 8
NPC = N // C              # 6250 nodes per core
NB = (NPC + 127) // 128   # 49 blocks
NPAD = NB * 128           # 6272
CH_A = 25                 # chunks 0..24 -> stream a
HALF_A = CH_A * 128       # 3200 nodes (a-half, incl none padded)
HALF_B = NPAD - HALF_A    # 3072 node slots (b-half, incl 22 pads)
NH_A = HALF_A * C         # 25600 rows in g_full_a
NH_B = HALF_B * C         # 24576 rows in g_full_b
SEG = int(os.environ.get("GCN_SEG", 2048))   # slots per dma_gather call (>1024 needs single_packet=False)
SEGC = SEG // 128         # chunks per segment
NG = 64                   # graphs

f32 = mybir.dt.float32
bf16 = mybir.dt.bfloat16
i16 = mybir.dt.int16
i32 = mybir.dt.int32
AT = mybir.AluOpType
ACTF = mybir.ActivationFunctionType
HDT = bf16 if H16 else f32


def _wrap16(vals: np.ndarray) -> np.ndarray:
    """slot i -> [i % 16, i // 16], replicated to 128 partitions."""
    n = len(vals)
    base = vals.astype(np.int16).reshape(n // 16, 16).T   # [16, n//16]
    return np.ascontiguousarray(np.tile(base, (8, 1)))


def build_wcat(W1, b1, W2, b2):
    import ml_dtypes
    wcat = np.zeros((128, NCONV * 128), np.float32)
    bcat = np.zeros((128, NCONV), np.float32)
    for cv in range(NCONV):
        l = (cv // 2) % L
        wcat[:, cv * 128:(cv + 1) * 128] = W1[l] if cv % 2 == 0 else W2[l]
        bcat[:, cv] = (b1[l] if cv % 2 == 0 else b2[l])
    if H16:
        wcat = wcat.astype(ml_dtypes.bfloat16)
    return np.ascontiguousarray(wcat), np.ascontiguousarray(bcat)


def host_prep(x, edge_index, batch):
    import ml_dtypes
    src = edge_index[0].astype(np.int64)
    dst = edge_index[1].astype(np.int64)
    deg = np.bincount(dst, minlength=N).astype(np.float64) + 1.0
    dis = (1.0 / np.sqrt(deg)).astype(np.float32)

    r = src // NPC
    k = src % NPC
    kb = k // 128
    kp = k % 128
    stream = (kb >= CH_A).astype(np.int64)              # 0 = a, 1 = b
    # chunk-major rows: a: r*3200 + p*25 + b ; b: r*3072 + p*24 + (b-25)
    loc = np.where(stream == 0,
                   r * HALF_A + kp * CH_A + kb,
                   r * HALF_B + kp * (NB - CH_A) + (kb - CH_A)).astype(np.int64)
    core_of = dst // NPC
    dloc = dst % NPC
    blk = dloc // 128
    id_in_blk = dloc % 128

    key = (core_of * 2 + stream) * NB + blk
    order = np.argsort(key, kind="stable")
    s_loc = loc[order]
    s_id = id_in_blk[order]
    gcnt = np.bincount(key, minlength=C * 2 * NB).reshape(C, 2, NB)
    goff = np.zeros(C * 2 * NB + 1, np.int64)
    np.cumsum(gcnt.reshape(-1), out=goff[1:])

    # shared chunks-per-block (max over cores), per stream
    CPB = [np.maximum.reduce(-(-gcnt[:, s, :] // 128), axis=0) for s in range(2)]
    chunk_blocks = [np.repeat(np.arange(NB), CPB[s]) for s in range(2)]
    S = [int(CPB[s].sum()) * 128 for s in range(2)]
    blk_chunk_start = [np.concatenate([[0], np.cumsum(CPB[s])]) for s in range(2)]

    idx_host = [[None] * C for _ in range(2)]
    ids_host = [[None] * C for _ in range(2)]
    for s in range(2):
        for c in range(C):
            ia = np.zeros(S[s], np.int64)
            da = np.full(S[s], -1.0, np.float32)
            for b in range(NB):
                g = (c * 2 + s) * NB + b
                cnt = goff[g + 1] - goff[g]
                o = int(blk_chunk_start[s][b]) * 128
                ia[o:o + cnt] = s_loc[goff[g]:goff[g + 1]]
                da[o:o + cnt] = s_id[goff[g]:goff[g + 1]]
            idx_host[s][c] = _wrap16(ia)
            ids_host[s][c] = np.ascontiguousarray(
                da.reshape(S[s] // 128, 128).T).astype(ml_dtypes.bfloat16)

    # segment calls: (slot_off, nslots, chunk0, nchunks)
    calls = []
    for s in range(2):
        cl = []
        off = 0
        while off < S[s]:
            n = min(SEG, S[s] - off)
            cl.append((off, n, off // 128, n // 128))
            off += n
        calls.append(cl)

    # per-core aux arrays
    dis_pad = np.zeros((C, NPAD), np.float32)
    for c in range(C):
        dis_pad[c, :NPC] = dis[c * NPC:(c + 1) * NPC]
    dis_nm = dis_pad.reshape(C, NB, 128).transpose(0, 2, 1).copy()   # [C,128,NB]
    dis_fm = np.repeat(dis_pad[:, None, :], 128, axis=1)             # [C,128,NPAD]

    x_fm = np.zeros((C, 128, NPAD), np.float32)
    for c in range(C):
        x_fm[c, :, :NPC] = x[c * NPC:(c + 1) * NPC].T
    if H16:
        x_fm = x_fm.astype(ml_dtypes.bfloat16)

    cnt = np.bincount(batch.astype(np.int64), minlength=NG).astype(np.float64)
    w = (1.0 / np.maximum(cnt, 1.0)).astype(np.float32)
    pool_mat = np.zeros((C, NPAD, NG), np.float32)
    for c in range(C):
        bl = batch[c * NPC:(c + 1) * NPC].astype(np.int64)
        pool_mat[c, np.arange(NPC), bl] = w[bl]
    pool_t = pool_mat.reshape(C, NB, 128, NG).transpose(0, 2, 1, 3).reshape(
        C, 128, NB * NG).copy()

    return dict(dis_nm=dis_nm, dis_fm=dis_fm, x_fm=x_fm, pool_t=pool_t,
                idx_host=idx_host, ids_host=ids_host, calls=calls,
                CPB=CPB, chunk_blocks=chunk_blocks, S=S,
                blk_chunk_start=blk_chunk_start)


def build_program(nc, st):
    calls = st["calls"]
    CPB = st["CPB"]
    blk_start = st["blk_chunk_start"]
    S = st["S"]

    # ---- I/O ----
    x_in = nc.dram_tensor("x_fm", [128, NPAD], HDT, kind="ExternalInput")
    w_in = nc.dram_tensor("wcat", [128, NCONV * 128], HDT, kind="ExternalInput")
    b_in = nc.dram_tensor("bcat", [128, NCONV], f32, kind="ExternalInput")
    dnm_in = nc.dram_tensor("dis_nm", [128, NB], f32, kind="ExternalInput")
    dfm_in = nc.dram_tensor("dis_fm", [128, NPAD], f32, kind="ExternalInput")
    pool_in = nc.dram_tensor("pool_t", [128, NB * NG], f32, kind="ExternalInput")
    ident_in = nc.dram_tensor("ident", [128, 128], f32, kind="ExternalInput")
    idx_in = [nc.dram_tensor(f"idx{s}", [128, S[s] // 16], i16,
                             kind="ExternalInput") for s in range(2)]
    ids_in = [nc.dram_tensor(f"ids{s}", [128, S[s] // 128], bf16,
                             kind="ExternalInput") for s in range(2)]
    out_t = nc.dram_tensor("out", [NG, 128], f32, kind="ExternalOutput")
    hdump_t = nc.dram_tensor("hdump", [128, NPAD], f32,
                             kind="ExternalOutput") if DUMP_H else None

    g_slice = [nc.dram_tensor("g_slice0", [HALF_A, 128], bf16, kind="Internal"),
               nc.dram_tensor("g_slice1", [HALF_B, 128], bf16, kind="Internal")]
    # double-buffered by conv parity: AG(i+1) never WAR-waits on conv i's
    # gathers
    g_full = [[nc.dram_tensor(f"g_full{s}_{p}", [NH_A if s == 0 else NH_B, 128],
                              bf16, kind="Internal", addr_space="Shared")
               for p in range(2)] for s in range(2)]
    ar_in = nc.dram_tensor("ar_in", [NG, 128], f32, kind="Internal")
    ar_out = nc.dram_tensor("ar_out", [NG, 128], f32, kind="Internal",
                            addr_space="Shared")
    rg = [list(range(C))]

    def ap3(t, off_elems, dims):
        return bass.AP(t, off_elems, dims)

    with tile.TileContext(nc) as tc:
        with tc.tile_pool(name="const", bufs=1) as cp, \
             tc.tile_pool(name="state", bufs=1) as sp, \
             tc.tile_pool(name="ph", bufs=2, space="PSUM") as php, \
             tc.tile_pool(name="pagg", bufs=int(os.environ.get("GCN_PAGG", 5)), space="PSUM") as pap:

            b_t = cp.tile([128, NCONV], f32, tag="b")
            dnm_t = cp.tile([128, NB], f32, tag="dnm")
            dfm_t = cp.tile([128, NPAD], f32, tag="dfm")
            ident_t = cp.tile([128, 128], f32, tag="ident")
            ident_bf = cp.tile([128, 128], bf16, tag="identbf")
            iota_seg_f = cp.tile([128, SEG], bf16, tag="iosegf")
            iota_pm_f = cp.tile([128, 128], f32, tag="iopmf")

            idx_res = [cp.tile([128, S[s] // 16], i16, tag=f"idxr{s}",
                               name=f"idxr{s}") for s in range(2)]
            ids_res = [cp.tile([128, S[s] // 128], bf16, tag=f"idsr{s}",
                               name=f"idsr{s}") for s in range(2)]
            h16 = sp.tile([128, NPAD], HDT, tag="h16")
            agg = sp.tile([128, NPAD], f32, tag="agg")
            g_nm = sp.tile([128, NPAD], bf16, tag="gnm")

            nc.sync.dma_start(b_t[:], b_in[:])
            nc.sync.dma_start(dnm_t[:], dnm_in[:])
            nc.sync.dma_start(dfm_t[:], dfm_in[:])
            nc.sync.dma_start(ident_t[:], ident_in[:])
            nc.vector.tensor_copy(ident_bf[:], ident_t[:])
            nc.sync.dma_start(h16[:], x_in[:])
            for s in range(2):
                nc.sync.dma_start(idx_res[s][:], idx_in[s][:])
                nc.sync.dma_start(ids_res[s][:], ids_in[s][:])
            nc.gpsimd.iota(iota_seg_f[:], pattern=[[0, SEGC], [1, 128]],
                           base=0, channel_multiplier=0,
                           allow_small_or_imprecise_dtypes=True)
            nc.gpsimd.iota(iota_pm_f[:], pattern=[[1, 128]], base=0,
                           channel_multiplier=-1,
                           allow_small_or_imprecise_dtypes=True)

            def bs(b):
                return slice(b * 128, (b + 1) * 128)

            def emit_g_dmas(half):
                """One contiguous DMA: g_nm cols -> chunk-major slice rows."""
                if SKIP_GDMA:
                    return
                if half == 0:
                    nc.sync.dma_start(
                        ap3(g_slice[0], 0, [[HALF_A, 128], [1, HALF_A]]),
                        g_nm[:, 0:HALF_A])
                else:
                    nc.sync.dma_start(
                        ap3(g_slice[1], 0, [[HALF_B, 128], [1, HALF_B]]),
                        g_nm[:, HALF_A:NPAD])

            def emit_ag(half, par):
                if SKIP_COLL:
                    return
                nc.gpsimd.collective_compute(
                    "AllGather", AT.bypass, replica_groups=rg,
                    ins=[g_slice[half][:]], outs=[g_full[half][par][:]])

            tailp = tc.alloc_tile_pool(name="tail", bufs=2)
            NBUF = int(os.environ.get("GCN_NBUF", 8))
            mp = tc.alloc_tile_pool(name="msg", bufs=NBUF)
            op = tc.alloc_tile_pool(name="oh", bufs=NBUF)
            tp = tc.alloc_tile_pool(name="meta", bufs=3)

            # ---- conv 0 warmup: h@W for all blocks, issue AGs ----
            w_t = tp.tile([128, 128], HDT, tag="wt", bufs=2)
            nc.sync.dma_start(w_t[:], w_in[:, 0:128])
            for b in range(NB):
                ph = php.tile([128, 128], f32, tag="ph")
                nc.tensor.matmul(ph[:], h16[:, bs(b)], w_t[:],
                                 start=True, stop=True)
                nc.scalar.activation(g_nm[:, bs(b)], ph[:], ACTF.Copy,
                                     scale=dnm_t[:, b:b + 1])
                if b == CH_A - 1:
                    emit_g_dmas(0)
                    emit_ag(0, 0)
            emit_g_dmas(1)
            emit_ag(1, 0)

            pool_tile = None
            ppool = None

            for cv in range(NCONV):
                last = (cv == NCONV - 1)
                par = cv % 2
                if last and TAILFOLD:
                    # prefetch pooling matrix; tail runs inside this conv's
                    # b-phase
                    pool_tile = tailp.tile([128, NB * NG], f32, tag="poolm",
                                           bufs=1)
                    nc.sync.dma_start(pool_tile[:], pool_in[:])
                    ppool = pap.tile([NG, 128], f32, tag="ppool", bufs=1)
                if AG_NODEP and not last:
                    # timing probe: issue next conv's AGs dependency-free
                    # (reads stale g_slice; results are WRONG)
                    emit_ag(0, (cv + 1) % 2)
                    emit_ag(1, (cv + 1) % 2)
                if not last:
                    w_nt = tp.tile([128, 128], HDT, tag="wt", bufs=2)
                    nc.sync.dma_start(w_nt[:],
                                      w_in[:, (cv + 1) * 128:(cv + 2) * 128])

                # per-stream segment state
                def make_stream(s):
                    msg_tiles = {}
                    oh_tiles = {}
                    state = {"emitted": -1}

                    def emit_seg(si):
                        off, n, c0, nch = calls[s][si]
                        msg = mp.tile([128, SEGC, 128], bf16, tag="msg")
                        if not SKIP_GATHER:
                            nc.gpsimd.dma_gather(
                                msg[:, :nch, :], g_full[s][par][:],
                                idx_res[s][:, off // 16:(off + n) // 16],
                                num_idxs=n, num_idxs_reg=n, elem_size=128,
                                single_packet=False,
                                queue_num=si % nc.num_swdge_queues)
                        else:
                            nc.vector.memset(msg[:, :nch, :], 0.0)
                        oh = op.tile([128, SEG], bf16, tag="oh")
                        sl = ids_res[s][:, c0:c0 + nch]
                        in1 = bass.AP(sl.tensor, sl.offset, sl.ap + [[0, 128]])
                        nc.vector.tensor_tensor(
                            oh[:].rearrange("p (c d) -> p c d", d=128)[:, :nch, :],
                            iota_seg_f[:].rearrange("p (c d) -> p c d", d=128)[:, :nch, :],
                            in1, AT.is_equal)
                        msg_tiles[si] = msg
                        oh_tiles[si] = oh

                    def get(ch):
                        si, jj = divmod(ch, SEGC)
                        while state["emitted"] < si:
                            state["emitted"] += 1
                            emit_seg(state["emitted"])
                        return (msg_tiles[si][:, jj, :],
                                oh_tiles[si][:, jj * 128:(jj + 1) * 128])

                    return get

                get_a = make_stream(0)
                get_b = make_stream(1)

                # ---- a-phase: self-loop start + stream-a chunks ----
                for b in range(NB):
                    pa = pap.tile([128, 128], f32, tag="pagg")
                    nch = int(CPB[0][b]) if not SKIP_STREAMS else 0
                    nc.tensor.matmul(pa[:], g_nm[:, bs(b)], ident_bf[:],
                                     start=True, stop=(nch == 0))
                    for j in range(nch):
                        m, o = get_a(int(blk_start[0][b]) + j)
                        nc.tensor.matmul(pa[:], m, o, start=False,
                                         stop=(j == nch - 1))
                    nc.scalar.activation(agg[:, bs(b)], pa[:], ACTF.Copy,
                                         scale=1.0)

                # ---- b-phase: stream-b chunks + fused epilogue +
                #      next conv h@W interleave ----
                for b in range(NB):
                    nch = int(CPB[1][b]) if not SKIP_STREAMS else 0
                    if nch > 0:
                        pb = pap.tile([128, 128], f32, tag="pagg")
                        for j in range(nch):
                            m, o = get_b(int(blk_start[1][b]) + j)
                            nc.tensor.matmul(pb[:], m, o, start=(j == 0),
                                             stop=(j == nch - 1))
                        nc.vector.tensor_tensor(agg[:, bs(b)], pb[:],
                                                agg[:, bs(b)], AT.add)
                    nc.vector.tensor_tensor(agg[:, bs(b)], agg[:, bs(b)],
                                            dfm_t[:, bs(b)], AT.mult)
                    actf = ACTF.Relu if cv % 2 == 0 else ACTF.Identity
                    # last conv: keep final h in f32 (in agg) for the tail
                    ep_dst = agg if last else h16
                    nc.scalar.activation(ep_dst[:, bs(b)], agg[:, bs(b)], actf,
                                         bias=b_t[:, cv:cv + 1], scale=1.0)
                    if last and TAILFOLD:
                        pt = php.tile([128, 128], f32, tag="ph")
                        nc.tensor.transpose(pt[:], agg[:, bs(b)], ident_t[:])
                        hb_t = tailp.tile([128, 128], f32, tag="hnmb")
                        nc.vector.tensor_copy(hb_t[:], pt[:])
                        nc.tensor.matmul(ppool[:],
                                         pool_tile[:, b * NG:(b + 1) * NG],
                                         hb_t[:], start=(b == 0),
                                         stop=(b == NB - 1))
                    if not last:
                        ph = php.tile([128, 128], f32, tag="ph")
                        nc.tensor.matmul(ph[:], h16[:, bs(b)], w_nt[:],
                                         start=True, stop=True)
                        nc.scalar.activation(g_nm[:, bs(b)], ph[:], ACTF.Copy,
                                             scale=dnm_t[:, b:b + 1])
                        if b == CH_A - 1:
                            if not AG_NODEP:
                                emit_g_dmas(0)
                                emit_ag(0, (cv + 1) % 2)
                if not last and not AG_NODEP:
                    emit_g_dmas(1)
                    emit_ag(1, (cv + 1) % 2)

            for p in (tp, op, mp):
                p.release()

            # ---- mean pool + AllReduce ----
            if DUMP_H:
                nc.sync.dma_start(hdump_t[:], agg[:])
            if not TAILFOLD:
                pool_tile = tailp.tile([128, NB * NG], f32, tag="poolm",
                                       bufs=1)
                nc.sync.dma_start(pool_tile[:], pool_in[:])
                hnm = tailp.tile([128, NPAD], f32, tag="hnm", bufs=1)
                for b in range(NB):
                    pt = php.tile([128, 128], f32, tag="ph")
                    nc.tensor.transpose(pt[:], agg[:, bs(b)], ident_t[:])
                    nc.vector.tensor_copy(hnm[:, bs(b)], pt[:])
                ppool = pap.tile([NG, 128], f32, tag="ppool", bufs=1)
                for b in range(NB):
                    nc.tensor.matmul(ppool[:],
                                     pool_tile[:, b * NG:(b + 1) * NG],
                                     hnm[:, bs(b)], start=(b == 0),
                                     stop=(b == NB - 1))
            pres = sp.tile([NG, 128], f32, tag="pres")
            nc.vector.tensor_copy(pres[:], ppool[:])
            nc.sync.dma_start(ar_in[:], pres[:])
            if not SKIP_COLL:
                nc.gpsimd.collective_compute(
                    "AllReduce", AT.add, replica_groups=rg,
                    ins=[ar_in[:]], outs=[ar_out[:]])
            ores = sp.tile([NG, 128], f32, tag="ores")
            nc.sync.dma_start(ores[:], ar_out[:] if not SKIP_COLL else ar_in[:])
            nc.sync.dma_start(out_t[:], ores[:])
            tailp.release()
    return nc


def kernel(x, edge_index, batch, W1, b1, W2, b2, _want_trace=False, _want_res=False):
    x = np.asarray(x)
    edge_index = np.asarray(edge_index)
    batch = np.asarray(batch)
    W1, b1, W2, b2 = (np.asarray(a) for a in (W1, b1, W2, b2))

    st = host_prep(x, edge_index, batch)
    wcat, bcat = build_wcat(W1, b1, W2, b2)

    nc = bacc.Bacc("TRN2", target_bir_lowering=False, debug=False,
                   enable_asserts=False, num_devices=C,
                   num_swdge_queues=int(os.environ.get("GCN_NQ", 3)))
    build_program(nc, st)
    nc.compile()

    ident = np.eye(128, dtype=np.float32)
    in_maps = []
    for c in range(C):
        in_maps.append({
            "x_fm": st["x_fm"][c],
            "wcat": wcat, "bcat": bcat,
            "dis_nm": st["dis_nm"][c], "dis_fm": st["dis_fm"][c],
            "pool_t": st["pool_t"][c], "ident": ident,
            "idx0": st["idx_host"][0][c], "idx1": st["idx_host"][1][c],
            "ids0": st["ids_host"][0][c], "ids1": st["ids_host"][1][c],
        })

    res = run_bass_kernel_spmd(nc, in_maps, core_ids=list(range(C)),
                               trace=_want_trace)
    out = res.results[0]["out"].astype(np.float32)
    if _want_trace or _want_res:
        return out, res
    return out



# revision 6
# speedup vs baseline: 1.1604x; 1.1604x over previous
"""GCN (8-layer, 16 GCNConv) on 8 TRN2 NeuronCores.

Strategy (v2 — software-pipelined across convs):
- dst-partition nodes across 8 cores (6250 each); weights replicated.
- norm separability: norm[e] = dis[src]*dis[dst], so each conv is
    g = dis * (h @ W)         (node-major, per-core slice, bf16)
    AllGather g (split into two half-collectives a/b)
    agg[f,d] = sum_e g_fm[src[e]] onehot[e,d]   via PE matmuls over
               128-edge chunks (msgs gathered edge-major by SWDGE dma_gather)
    h' = relu?(dis * agg + b)  (feature-major)
- self-loop folded in as the START matmul of each block's a-phase PSUM
  accumulation chain (g_nm block x ident), so agg is written once per
  phase: a-phase via scalar-engine activation copy (PSUM->SBUF), b-phase
  via one DVE add.
- per-block epilogue fused into the b-phase loop, immediately followed by
  the NEXT conv's h@W matmul + g_nm scale for that block, so the next
  conv's half-AllGathers are issued while the current conv's scatter is
  still draining (cross-conv software pipeline).
- h is stored bf16 (h16); agg accumulates f32; matmuls run bf16.
- edges are host-sorted by (stream, dst block); per-block chunk counts are
  shared across cores (max), pad slots gather row 0 with onehot id -1.
- int16 gather indices: the a/b split keeps indices < 25600.
- final mean-pool via matmul with host-built pooling matrix + AllReduce.
"""
import numpy as np
import concourse.bass as bass
import concourse.mybir as mybir
import concourse.bacc as bacc
import concourse.tile as tile
from concourse.bass_utils import run_bass_kernel_spmd

import os
N = 50000
E = 600000
D = 128
L = 8
NCONV = int(os.environ.get("GCN_NCONV", 2 * L))
EARLY_DUMP = os.environ.get("GCN_EARLY_DUMP", "1") == "1"
PREF_A = int(os.environ.get("GCN_PREF_A", 4))
DVE_GSCALE = os.environ.get("GCN_DVE_GSCALE", "1") == "1"
DVE_ACOPY = os.environ.get("GCN_DVE_ACOPY", "") == "1"
SKIP_COLL = os.environ.get("GCN_SKIP_COLL", "") == "1"
SKIP_GDMA = os.environ.get("GCN_SKIP_GDMA", "") == "1"
SKIP_STREAMS = os.environ.get("GCN_SKIP_STREAMS", "") == "1"
SKIP_GATHER = os.environ.get("GCN_SKIP_GATHER", "") == "1"
DUMP_H = os.environ.get("GCN_DUMP_H", "") == "1"
H16 = os.environ.get("GCN_H16", "1") == "1"
AG_NODEP = os.environ.get("GCN_AG_NODEP", "") == "1"
TAILFOLD = os.environ.get("GCN_TAILFOLD", "1") == "1"
C = 8
NPC = N // C              # 6250 nodes per core
NB = (NPC + 127) // 128   # 49 blocks
NPAD = NB * 128           # 6272
CH_A = 25                 # chunks 0..24 -> stream a
HALF_A = CH_A * 128       # 3200 nodes (a-half, incl none padded)
HALF_B = NPAD - HALF_A    # 3072 node slots (b-half, incl 22 pads)
NH_A = HALF_A * C         # 25600 rows in g_full_a
NH_B = HALF_B * C         # 24576 rows in g_full_b
SEG = int(os.environ.get("GCN_SEG", 1024))   # slots per dma_gather call (>1024 needs single_packet=False)
SEGC = SEG // 128         # chunks per segment
NG = 64                   # graphs

f32 = mybir.dt.float32
bf16 = mybir.dt.bfloat16
i16 = mybir.dt.int16
i32 = mybir.dt.int32
AT = mybir.AluOpType
ACTF = mybir.ActivationFunctionType
HDT = bf16 if H16 else f32


def _wrap16(vals: np.ndarray) -> np.ndarray:
    """slot i -> [i % 16, i // 16], replicated to 128 partitions."""
    n = len(vals)
    base = vals.astype(np.int16).reshape(n // 16, 16).T   # [16, n//16]
    return np.ascontiguousarray(np.tile(base, (8, 1)))


def build_wcat(W1, b1, W2, b2):
    import ml_dtypes
    wcat = np.zeros((128, NCONV * 128), np.float32)
    bcat = np.zeros((128, NCONV), np.float32)
    for cv in range(NCONV):
        l = (cv // 2) % L
        wcat[:, cv * 128:(cv + 1) * 128] = W1[l] if cv % 2 == 0 else W2[l]
        bcat[:, cv] = (b1[l] if cv % 2 == 0 else b2[l])
    if H16:
        wcat = wcat.astype(ml_dtypes.bfloat16)
    return np.ascontiguousarray(wcat), np.ascontiguousarray(bcat)


def host_prep(x, edge_index, batch):
    import ml_dtypes
    src = edge_index[0].astype(np.int64)
    dst = edge_index[1].astype(np.int64)
    deg = np.bincount(dst, minlength=N).astype(np.float64) + 1.0
    dis = (1.0 / np.sqrt(deg)).astype(np.float32)

    r = src // NPC
    k = src % NPC
    kb = k // 128
    kp = k % 128
    stream = (kb >= CH_A).astype(np.int64)              # 0 = a, 1 = b
    # chunk-major rows: a: r*3200 + p*25 + b ; b: r*3072 + p*24 + (b-25)
    loc = np.where(stream == 0,
                   r * HALF_A + kp * CH_A + kb,
                   r * HALF_B + kp * (NB - CH_A) + (kb - CH_A)).astype(np.int64)
    core_of = dst // NPC
    dloc = dst % NPC
    blk = dloc // 128
    id_in_blk = dloc % 128

    key = (core_of * 2 + stream) * NB + blk
    order = np.argsort(key, kind="stable")
    s_loc = loc[order]
    s_id = id_in_blk[order]
    gcnt = np.bincount(key, minlength=C * 2 * NB).reshape(C, 2, NB)
    goff = np.zeros(C * 2 * NB + 1, np.int64)
    np.cumsum(gcnt.reshape(-1), out=goff[1:])

    # shared chunks-per-block (max over cores), per stream
    CPB = [np.maximum.reduce(-(-gcnt[:, s, :] // 128), axis=0) for s in range(2)]
    chunk_blocks = [np.repeat(np.arange(NB), CPB[s]) for s in range(2)]
    S = [int(CPB[s].sum()) * 128 for s in range(2)]
    blk_chunk_start = [np.concatenate([[0], np.cumsum(CPB[s])]) for s in range(2)]

    idx_host = [[None] * C for _ in range(2)]
    ids_host = [[None] * C for _ in range(2)]
    for s in range(2):
        for c in range(C):
            ia = np.zeros(S[s], np.int64)
            da = np.full(S[s], -1.0, np.float32)
            for b in range(NB):
                g = (c * 2 + s) * NB + b
                cnt = goff[g + 1] - goff[g]
                o = int(blk_chunk_start[s][b]) * 128
                ia[o:o + cnt] = s_loc[goff[g]:goff[g + 1]]
                da[o:o + cnt] = s_id[goff[g]:goff[g + 1]]
            idx_host[s][c] = _wrap16(ia)
            ids_host[s][c] = np.ascontiguousarray(
                da.reshape(S[s] // 128, 128).T).astype(ml_dtypes.bfloat16)

    # segment calls: (slot_off, nslots, chunk0, nchunks)
    calls = []
    for s in range(2):
        cl = []
        off = 0
        while off < S[s]:
            n = min(SEG, S[s] - off)
            cl.append((off, n, off // 128, n // 128))
            off += n
        calls.append(cl)

    # per-core aux arrays
    dis_pad = np.zeros((C, NPAD), np.float32)
    for c in range(C):
        dis_pad[c, :NPC] = dis[c * NPC:(c + 1) * NPC]
    dis_nm = dis_pad.reshape(C, NB, 128).transpose(0, 2, 1).copy()   # [C,128,NB]
    dis_fm = np.repeat(dis_pad[:, None, :], 128, axis=1)             # [C,128,NPAD]

    x_fm = np.zeros((C, 128, NPAD), np.float32)
    for c in range(C):
        x_fm[c, :, :NPC] = x[c * NPC:(c + 1) * NPC].T
    if H16:
        x_fm = x_fm.astype(ml_dtypes.bfloat16)

    cnt = np.bincount(batch.astype(np.int64), minlength=NG).astype(np.float64)
    w = (1.0 / np.maximum(cnt, 1.0)).astype(np.float32)
    pool_mat = np.zeros((C, NPAD, NG), np.float32)
    for c in range(C):
        bl = batch[c * NPC:(c + 1) * NPC].astype(np.int64)
        pool_mat[c, np.arange(NPC), bl] = w[bl]
    pool_t = pool_mat.reshape(C, NB, 128, NG).transpose(0, 2, 1, 3).reshape(
        C, 128, NB * NG).copy()

    return dict(dis_nm=dis_nm, dis_fm=dis_fm, x_fm=x_fm, pool_t=pool_t,
                idx_host=idx_host, ids_host=ids_host, calls=calls,
                CPB=CPB, chunk_blocks=chunk_blocks, S=S,
                blk_chunk_start=blk_chunk_start)


def build_program(nc, st):
    calls = st["calls"]
    CPB = st["CPB"]
    blk_start = st["blk_chunk_start"]
    S = st["S"]

    # ---- I/O ----
    x_in = nc.dram_tensor("x_fm", [128, NPAD], HDT, kind="ExternalInput")
    w_in = nc.dram_tensor("wcat", [128, NCONV * 128], HDT, kind="ExternalInput")
    b_in = nc.dram_tensor("bcat", [128, NCONV], f32, kind="ExternalInput")
    dnm_in = nc.dram_tensor("dis_nm", [128, NB], f32, kind="ExternalInput")
    dfm_in = nc.dram_tensor("dis_fm", [128, NPAD], f32, kind="ExternalInput")
    pool_in = nc.dram_tensor("pool_t", [128, NB * NG], f32, kind="ExternalInput")
    ident_in = nc.dram_tensor("ident", [128, 128], f32, kind="ExternalInput")
    idx_in = [nc.dram_tensor(f"idx{s}", [128, S[s] // 16], i16,
                             kind="ExternalInput") for s in range(2)]
    ids_in = [nc.dram_tensor(f"ids{s}", [128, S[s] // 128], bf16,
                             kind="ExternalInput") for s in range(2)]
    out_t = nc.dram_tensor("out", [NG, 128], f32, kind="ExternalOutput")
    hdump_t = nc.dram_tensor("hdump", [128, NPAD], f32,
                             kind="ExternalOutput") if DUMP_H else None

    g_slice = [nc.dram_tensor("g_slice0", [HALF_A, 128], bf16, kind="Internal"),
               nc.dram_tensor("g_slice1", [HALF_B, 128], bf16, kind="Internal")]
    # double-buffered by conv parity: AG(i+1) never WAR-waits on conv i's
    # gathers
    g_full = [[nc.dram_tensor(f"g_full{s}_{p}", [NH_A if s == 0 else NH_B, 128],
                              bf16, kind="Internal", addr_space="Shared")
               for p in range(2)] for s in range(2)]
    ar_in = nc.dram_tensor("ar_in", [NG, 128], f32, kind="Internal")
    ar_out = nc.dram_tensor("ar_out", [NG, 128], f32, kind="Internal",
                            addr_space="Shared")
    rg = [list(range(C))]

    def ap3(t, off_elems, dims):
        return bass.AP(t, off_elems, dims)

    with tile.TileContext(nc) as tc:
        with tc.tile_pool(name="const", bufs=1) as cp, \
             tc.tile_pool(name="state", bufs=1) as sp, \
             tc.tile_pool(name="ph", bufs=2, space="PSUM") as php, \
             tc.tile_pool(name="pagg", bufs=int(os.environ.get("GCN_PAGG", 5)), space="PSUM") as pap:

            b_t = cp.tile([128, NCONV], f32, tag="b")
            dnm_t = cp.tile([128, NB], f32, tag="dnm")
            dfm_t = cp.tile([128, NPAD], f32, tag="dfm")
            ident_t = cp.tile([128, 128], f32, tag="ident")
            ident_bf = cp.tile([128, 128], bf16, tag="identbf")
            iota_seg_f = cp.tile([128, SEG], bf16, tag="iosegf")
            iota_pm_f = cp.tile([128, 128], f32, tag="iopmf")

            idx_res = [cp.tile([128, S[s] // 16], i16, tag=f"idxr{s}",
                               name=f"idxr{s}") for s in range(2)]
            ids_res = [cp.tile([128, S[s] // 128], bf16, tag=f"idsr{s}",
                               name=f"idsr{s}") for s in range(2)]
            h16 = sp.tile([128, NPAD], HDT, tag="h16")
            agg = sp.tile([128, NPAD], f32, tag="agg")
            g_nm = sp.tile([128, NPAD], bf16, tag="gnm")

            nc.sync.dma_start(b_t[:], b_in[:])
            nc.sync.dma_start(dnm_t[:], dnm_in[:])
            nc.sync.dma_start(dfm_t[:], dfm_in[:])
            nc.sync.dma_start(ident_t[:], ident_in[:])
            nc.vector.tensor_copy(ident_bf[:], ident_t[:])
            nc.sync.dma_start(h16[:], x_in[:])
            for s in range(2):
                nc.sync.dma_start(idx_res[s][:], idx_in[s][:])
                nc.sync.dma_start(ids_res[s][:], ids_in[s][:])
            nc.gpsimd.iota(iota_seg_f[:], pattern=[[0, SEGC], [1, 128]],
                           base=0, channel_multiplier=0,
                           allow_small_or_imprecise_dtypes=True)
            nc.gpsimd.iota(iota_pm_f[:], pattern=[[1, 128]], base=0,
                           channel_multiplier=-1,
                           allow_small_or_imprecise_dtypes=True)

            def bs(b):
                return slice(b * 128, (b + 1) * 128)

            def emit_g_dump(col0, col1):
                """Dump g_nm[:, col0:col1) into its slice rows (one stream)."""
                if SKIP_GDMA:
                    return
                half = 0 if col0 < HALF_A else 1
                base = 0 if half == 0 else HALF_A
                hsz = HALF_A if half == 0 else HALF_B
                nc.sync.dma_start(
                    ap3(g_slice[half], col0 - base, [[hsz, 128], [1, col1 - col0]]),
                    g_nm[:, col0:col1])

            def emit_g_dmas(half):
                if half == 0:
                    emit_g_dump(0, HALF_A)
                else:
                    emit_g_dump(HALF_A, NPAD)

            def emit_ag(half, par):
                if SKIP_COLL:
                    return
                nc.gpsimd.collective_compute(
                    "AllGather", AT.bypass, replica_groups=rg,
                    ins=[g_slice[half][:]], outs=[g_full[half][par][:]])

            tailp = tc.alloc_tile_pool(name="tail", bufs=2)
            NBUF = int(os.environ.get("GCN_NBUF", 8))
            mp = tc.alloc_tile_pool(name="msg", bufs=NBUF)
            op = tc.alloc_tile_pool(name="oh", bufs=NBUF)
            tp = tc.alloc_tile_pool(name="meta", bufs=3)

            # per-stream segment state (parameterized by conv parity)
            def make_stream(s, par):
                msg_tiles = {}
                oh_tiles = {}
                state = {"emitted": -1}

                def emit_seg(si):
                    off, n, c0, nch = calls[s][si]
                    msg = mp.tile([128, SEGC, 128], bf16, tag="msg")
                    if not SKIP_GATHER:
                        nc.gpsimd.dma_gather(
                            msg[:, :nch, :], g_full[s][par][:],
                            idx_res[s][:, off // 16:(off + n) // 16],
                            num_idxs=n, num_idxs_reg=n, elem_size=128,
                            single_packet=False,
                            queue_num=si % nc.num_swdge_queues)
                    else:
                        nc.vector.memset(msg[:, :nch, :], 0.0)
                    oh = op.tile([128, SEG], bf16, tag="oh")
                    sl = ids_res[s][:, c0:c0 + nch]
                    in1 = bass.AP(sl.tensor, sl.offset, sl.ap + [[0, 128]])
                    nc.vector.tensor_tensor(
                        oh[:].rearrange("p (c d) -> p c d", d=128)[:, :nch, :],
                        iota_seg_f[:].rearrange("p (c d) -> p c d", d=128)[:, :nch, :],
                        in1, AT.is_equal)
                    msg_tiles[si] = msg
                    oh_tiles[si] = oh

                def prefetch(nseg):
                    while state["emitted"] < nseg - 1:
                        state["emitted"] += 1
                        emit_seg(state["emitted"])

                def get(ch):
                    si, jj = divmod(ch, SEGC)
                    while state["emitted"] < si:
                        state["emitted"] += 1
                        emit_seg(state["emitted"])
                    return (msg_tiles[si][:, jj, :],
                            oh_tiles[si][:, jj * 128:(jj + 1) * 128])

                return get, prefetch

            def emit_gscale(b, ph):
                """g_nm[:, bs(b)] = ph * dis_nm[:, b] (broadcast col)."""
                if DVE_GSCALE:
                    sl = dnm_t[:, b:b + 1]
                    in1 = bass.AP(sl.tensor, sl.offset,
                                  [list(sl.ap[0]), [0, 128]])
                    nc.vector.tensor_tensor(g_nm[:, bs(b)], ph[:], in1,
                                            AT.mult)
                else:
                    nc.scalar.activation(g_nm[:, bs(b)], ph[:], ACTF.Copy,
                                         scale=dnm_t[:, b:b + 1])

            # dump boundaries: halves of each stream-half (block index after
            # whose epilogue the g_nm column range is complete)
            DUMPS = {CH_A // 2: (0, (CH_A // 2 + 1) * 128),
                     CH_A - 1: ((CH_A // 2 + 1) * 128, HALF_A),
                     CH_A + (NB - CH_A) // 2: (HALF_A, (CH_A + (NB - CH_A) // 2 + 1) * 128),
                     NB - 1: ((CH_A + (NB - CH_A) // 2 + 1) * 128, NPAD)}

            # ---- conv 0 warmup: h@W for all blocks, issue AGs ----
            w_t = tp.tile([128, 128], HDT, tag="wt", bufs=2)
            nc.sync.dma_start(w_t[:], w_in[:, 0:128])
            for b in range(NB):
                ph = php.tile([128, 128], f32, tag="ph")
                nc.tensor.matmul(ph[:], h16[:, bs(b)], w_t[:],
                                 start=True, stop=True)
                emit_gscale(b, ph)
                if EARLY_DUMP and b in DUMPS:
                    emit_g_dump(*DUMPS[b])
                if b == CH_A - 1:
                    if not EARLY_DUMP:
                        emit_g_dmas(0)
                    emit_ag(0, 0)
            if not EARLY_DUMP:
                emit_g_dmas(1)
            emit_ag(1, 0)

            pool_tile = None
            ppool = None
            streams = (make_stream(0, 0), make_stream(1, 0))

            for cv in range(NCONV):
                last = (cv == NCONV - 1)
                par = cv % 2
                if last and TAILFOLD:
                    # prefetch pooling matrix; tail runs inside this conv's
                    # b-phase
                    pool_tile = tailp.tile([128, NB * NG], f32, tag="poolm",
                                           bufs=1)
                    nc.sync.dma_start(pool_tile[:], pool_in[:])
                    ppool = pap.tile([NG, 128], f32, tag="ppool", bufs=1)
                if AG_NODEP and not last:
                    # timing probe: issue next conv's AGs dependency-free
                    # (reads stale g_slice; results are WRONG)
                    emit_ag(0, (cv + 1) % 2)
                    emit_ag(1, (cv + 1) % 2)
                if not last:
                    w_nt = tp.tile([128, 128], HDT, tag="wt", bufs=2)
                    nc.sync.dma_start(w_nt[:],
                                      w_in[:, (cv + 1) * 128:(cv + 2) * 128])

                (get_a, _), (get_b, _) = streams

                # ---- a-phase: self-loop start + stream-a chunks ----
                for b in range(NB):
                    pa = pap.tile([128, 128], f32, tag="pagg")
                    nch = int(CPB[0][b]) if not SKIP_STREAMS else 0
                    nc.tensor.matmul(pa[:], g_nm[:, bs(b)], ident_bf[:],
                                     start=True, stop=(nch == 0))
                    for j in range(nch):
                        m, o = get_a(int(blk_start[0][b]) + j)
                        nc.tensor.matmul(pa[:], m, o, start=False,
                                         stop=(j == nch - 1))
                    nc.scalar.activation(agg[:, bs(b)], pa[:], ACTF.Copy,
                                         scale=1.0)

                # ---- b-phase: stream-b chunks + fused epilogue +
                #      next conv h@W interleave ----
                for b in range(NB):
                    nch = int(CPB[1][b]) if not SKIP_STREAMS else 0
                    if nch > 0:
                        pb = pap.tile([128, 128], f32, tag="pagg")
                        for j in range(nch):
                            m, o = get_b(int(blk_start[1][b]) + j)
                            nc.tensor.matmul(pb[:], m, o, start=(j == 0),
                                             stop=(j == nch - 1))
                        nc.vector.tensor_tensor(agg[:, bs(b)], pb[:],
                                                agg[:, bs(b)], AT.add)
                    nc.vector.tensor_tensor(agg[:, bs(b)], agg[:, bs(b)],
                                            dfm_t[:, bs(b)], AT.mult)
                    actf = ACTF.Relu if cv % 2 == 0 else ACTF.Identity
                    # last conv: keep final h in f32 (in agg) for the tail
                    ep_dst = agg if last else h16
                    nc.scalar.activation(ep_dst[:, bs(b)], agg[:, bs(b)], actf,
                                         bias=b_t[:, cv:cv + 1], scale=1.0)
                    if last and TAILFOLD:
                        pt = php.tile([128, 128], f32, tag="ph")
                        nc.tensor.transpose(pt[:], agg[:, bs(b)], ident_t[:])
                        hb_t = tailp.tile([128, 128], f32, tag="hnmb")
                        nc.vector.tensor_copy(hb_t[:], pt[:])
                        nc.tensor.matmul(ppool[:],
                                         pool_tile[:, b * NG:(b + 1) * NG],
                                         hb_t[:], start=(b == 0),
                                         stop=(b == NB - 1))
                    if not last:
                        ph = php.tile([128, 128], f32, tag="ph")
                        nc.tensor.matmul(ph[:], h16[:, bs(b)], w_nt[:],
                                         start=True, stop=True)
                        emit_gscale(b, ph)
                        if EARLY_DUMP and b in DUMPS:
                            emit_g_dump(*DUMPS[b])
                        if b == CH_A - 1:
                            if not AG_NODEP:
                                if not EARLY_DUMP:
                                    emit_g_dmas(0)
                                emit_ag(0, (cv + 1) % 2)
                if not last:
                    nstreams = (make_stream(0, (cv + 1) % 2),
                                make_stream(1, (cv + 1) % 2))
                    if not AG_NODEP:
                        if not EARLY_DUMP:
                            emit_g_dmas(1)
                        if PREF_A > 0 and not SKIP_STREAMS:
                            # emit a few next-conv stream-a gathers before the
                            # AG_b trigger so its input-ready wait overlaps
                            # with their descriptor generation
                            nstreams[0][1](min(PREF_A, len(calls[0])))
                        emit_ag(1, (cv + 1) % 2)
                    streams = nstreams

            for p in (tp, op, mp):
                p.release()

            # ---- mean pool + AllReduce ----
            if DUMP_H:
                nc.sync.dma_start(hdump_t[:], agg[:])
            if not TAILFOLD:
                pool_tile = tailp.tile([128, NB * NG], f32, tag="poolm",
                                       bufs=1)
                nc.sync.dma_start(pool_tile[:], pool_in[:])
                hnm = tailp.tile([128, NPAD], f32, tag="hnm", bufs=1)
                for b in range(NB):
                    pt = php.tile([128, 128], f32, tag="ph")
                    nc.tensor.transpose(pt[:], agg[:, bs(b)], ident_t[:])
                    nc.vector.tensor_copy(hnm[:, bs(b)], pt[:])
                ppool = pap.tile([NG, 128], f32, tag="ppool", bufs=1)
                for b in range(NB):
                    nc.tensor.matmul(ppool[:],
                                     pool_tile[:, b * NG:(b + 1) * NG],
                                     hnm[:, bs(b)], start=(b == 0),
                                     stop=(b == NB - 1))
            pres = sp.tile([NG, 128], f32, tag="pres")
            nc.vector.tensor_copy(pres[:], ppool[:])
            nc.sync.dma_start(ar_in[:], pres[:])
            if not SKIP_COLL:
                nc.gpsimd.collective_compute(
                    "AllReduce", AT.add, replica_groups=rg,
                    ins=[ar_in[:]], outs=[ar_out[:]])
            ores = sp.tile([NG, 128], f32, tag="ores")
            nc.sync.dma_start(ores[:], ar_out[:] if not SKIP_COLL else ar_in[:])
            nc.sync.dma_start(out_t[:], ores[:])
            tailp.release()
    return nc


def kernel(x, edge_index, batch, W1, b1, W2, b2, _want_trace=False, _want_res=False):
    x = np.asarray(x)
    edge_index = np.asarray(edge_index)
    batch = np.asarray(batch)
    W1, b1, W2, b2 = (np.asarray(a) for a in (W1, b1, W2, b2))

    st = host_prep(x, edge_index, batch)
    wcat, bcat = build_wcat(W1, b1, W2, b2)

    nc = bacc.Bacc("TRN2", target_bir_lowering=False, debug=False,
                   enable_asserts=False, num_devices=C,
                   num_swdge_queues=int(os.environ.get("GCN_NQ", 3)))
    build_program(nc, st)
    nc.compile()

    ident = np.eye(128, dtype=np.float32)
    in_maps = []
    for c in range(C):
        in_maps.append({
            "x_fm": st["x_fm"][c],
            "wcat": wcat, "bcat": bcat,
            "dis_nm": st["dis_nm"][c], "dis_fm": st["dis_fm"][c],
            "pool_t": st["pool_t"][c], "ident": ident,
            "idx0": st["idx_host"][0][c], "idx1": st["idx_host"][1][c],
            "ids0": st["ids_host"][0][c], "ids1": st["ids_host"][1][c],
        })

    res = run_bass_kernel_spmd(nc, in_maps, core_ids=list(range(C)),
                               trace=_want_trace)
    out = res.results[0]["out"].astype(np.float32)
    if _want_trace or _want_res:
        return out, res
    return out



# revision 13
# speedup vs baseline: 2.2400x; 1.9303x over previous
"""GCN (8-layer, 16 GCNConv) on 8 TRN2 NeuronCores.

Strategy (v2 — software-pipelined across convs):
- dst-partition nodes across 8 cores (6250 each); weights replicated.
- norm separability: norm[e] = dis[src]*dis[dst], so each conv is
    g = dis * (h @ W)         (node-major, per-core slice, bf16)
    AllGather g (split into two half-collectives a/b)
    agg[f,d] = sum_e g_fm[src[e]] onehot[e,d]   via PE matmuls over
               128-edge chunks (msgs gathered edge-major by SWDGE dma_gather)
    h' = relu?(dis * agg + b)  (feature-major)
- self-loop folded in as the START matmul of each block's a-phase PSUM
  accumulation chain (g_nm block x ident), so agg is written once per
  phase: a-phase via scalar-engine activation copy (PSUM->SBUF), b-phase
  via one DVE add.
- per-block epilogue fused into the b-phase loop, immediately followed by
  the NEXT conv's h@W matmul + g_nm scale for that block, so the next
  conv's half-AllGathers are issued while the current conv's scatter is
  still draining (cross-conv software pipeline).
- h is stored bf16 (h16); agg accumulates f32; matmuls run bf16.
- edges are host-sorted by (stream, dst block); per-block chunk counts are
  shared across cores (max), pad slots gather row 0 with onehot id -1.
- int16 gather indices: the a/b split keeps indices < 25600.
- final mean-pool via matmul with host-built pooling matrix + AllReduce.
"""
import numpy as np
import concourse.bass as bass
import concourse.mybir as mybir
import concourse.bacc as bacc
import concourse.tile as tile
from concourse.bass_utils import run_bass_kernel_spmd

import os
N = 50000
E = 600000
D = 128
L = 8
NCONV = int(os.environ.get("GCN_NCONV", 2 * L))
EARLY_DUMP = os.environ.get("GCN_EARLY_DUMP", "1") == "1"
PREF_A = int(os.environ.get("GCN_PREF_A", 4))
DVE_GSCALE = os.environ.get("GCN_DVE_GSCALE", "1") == "1"
DVE_ACOPY = os.environ.get("GCN_DVE_ACOPY", "") == "1"
DUMMY_COLL = int(os.environ.get("GCN_DUMMY_COLL", 0))
SKIP_COLL = os.environ.get("GCN_SKIP_COLL", "") == "1"
SKIP_GDMA = os.environ.get("GCN_SKIP_GDMA", "") == "1"
SKIP_STREAMS = os.environ.get("GCN_SKIP_STREAMS", "") == "1"
SKIP_GATHER = os.environ.get("GCN_SKIP_GATHER", "") == "1"
DUMP_H = os.environ.get("GCN_DUMP_H", "") == "1"
H16 = os.environ.get("GCN_H16", "1") == "1"
AG_NODEP = os.environ.get("GCN_AG_NODEP", "") == "1"
TAILFOLD = os.environ.get("GCN_TAILFOLD", "1") == "1"
C = 8
BALANCE = os.environ.get("GCN_BALANCE", "1") == "1"
NPC = N // C              # 6250 nodes per core
NB = 50 if BALANCE else (NPC + 127) // 128   # blocks (128 slots each)
NPAD = NB * 128           # 6400 / 6272
CH_A = 25                 # chunks 0..24 -> stream a
HALF_A = CH_A * 128       # 3200 nodes (a-half)
HALF_B = NPAD - HALF_A    # 3200 / 3072 node slots (b-half)
NH_A = HALF_A * C         # 25600 rows in g_full_a
NH_B = HALF_B * C         # 25600 / 24576 rows in g_full_b
SEG = int(os.environ.get("GCN_SEG", 1024))   # slots per dma_gather call (>1024 needs single_packet=False)
SEGC = SEG // 128         # chunks per segment
NG = 64                   # graphs

f32 = mybir.dt.float32
bf16 = mybir.dt.bfloat16
i16 = mybir.dt.int16
i32 = mybir.dt.int32
AT = mybir.AluOpType
ACTF = mybir.ActivationFunctionType
HDT = bf16 if H16 else f32


def _wrap16(vals: np.ndarray) -> np.ndarray:
    """slot i -> [i % 16, i // 16], replicated to 128 partitions."""
    n = len(vals)
    base = vals.astype(np.int16).reshape(n // 16, 16).T   # [16, n//16]
    return np.ascontiguousarray(np.tile(base, (8, 1)))


def build_wcat(W1, b1, W2, b2):
    import ml_dtypes
    wcat = np.zeros((128, NCONV * 128), np.float32)
    bcat = np.zeros((128, NCONV), np.float32)
    for cv in range(NCONV):
        l = (cv // 2) % L
        wcat[:, cv * 128:(cv + 1) * 128] = W1[l] if cv % 2 == 0 else W2[l]
        bcat[:, cv] = (b1[l] if cv % 2 == 0 else b2[l])
    if H16:
        wcat = wcat.astype(ml_dtypes.bfloat16)
    return np.ascontiguousarray(wcat), np.ascontiguousarray(bcat)


def _balanced_slots(src, dst):
    """Assign nodes to (core, block, pos) slots so each (core, block, stream)
    bin's in-edge count fits 6 chunks of 128. Returns slot_of[v]."""
    NPB = NPC // NB                       # 125 real nodes per block
    NBINS = C * NB
    CAP = 6 * 128
    deg_in = np.bincount(dst, minlength=N).astype(np.int64)

    # LPT: fill 400 bins (capacity NPB) balancing total in-degree
    import heapq
    order = np.argsort(-deg_in, kind="stable")
    heap = [(0, b) for b in range(NBINS)]
    heapq.heapify(heap)
    counts = np.zeros(NBINS, np.int64)
    bin_of = np.zeros(N, np.int64)
    rank_in_bin = np.zeros(N, np.int64)
    for v in order:
        while True:
            s, b = heapq.heappop(heap)
            if counts[b] < NPB:
                break
        bin_of[v] = b
        rank_in_bin[v] = counts[b]
        counts[b] += 1
        if counts[b] < NPB:
            heapq.heappush(heap, (s + int(deg_in[v]), b))

    # stream split: choose 25 a-side blocks per core so per-bin stream
    # in-counts stay <= CAP (local search over block pair swaps)
    M = np.zeros((NBINS, NBINS), np.int64)
    np.add.at(M, (bin_of[src], bin_of[dst]), 1)
    T = M.sum(axis=0)
    def viol(a):
        return (np.maximum(a - CAP, 0) + np.maximum(T - a - CAP, 0)).sum()

    best_side, best_cur = None, None
    for seed in range(4):
        rng = np.random.default_rng(seed)
        side = np.zeros(NBINS, bool)
        for c in range(C):
            pick = rng.permutation(NB)[:NB - CH_A]
            side[c * NB + pick] = True            # True = b-side
        A = M[~side].sum(axis=0)
        cur = viol(A)
        stall = 0
        for it in range(300 * C):
            if cur == 0 or stall > 3 * C:
                break
            c = it % C
            rows = np.arange(c * NB, (c + 1) * NB)
            ar = rows[~side[rows]]
            br = rows[side[rows]]
            delta = M[br][:, None, :] - M[ar][None, :, :]     # [b, a, NBINS]
            newA = A[None, None, :] + delta
            v = (np.maximum(newA - CAP, 0)
                 + np.maximum(T[None, None, :] - newA - CAP, 0)).sum(axis=2)
            # sideways moves allowed (random among ties) to escape plateaus
            vmin = v.min()
            if vmin > cur:
                stall += 1
                continue
            stall = stall + 1 if vmin == cur else 0
            cands = np.argwhere(v == vmin)
            bi, ai = cands[rng.integers(len(cands))]
            cur = int(vmin)
            A = A + M[br[bi]] - M[ar[ai]]
            side[br[bi]] = False
            side[ar[ai]] = True
        if best_cur is None or cur < best_cur:
            best_cur, best_side = cur, side.copy()
        if best_cur == 0:
            break
    side = best_side
    A = M[~side].sum(axis=0)

    # chunk position of each block: a-side -> 0..24, b-side -> 25..49
    pos_of_bin = np.zeros(NBINS, np.int64)
    for c in range(C):
        rows = np.arange(c * NB, (c + 1) * NB)
        pos_of_bin[rows[~side[rows]]] = np.arange(CH_A)
        pos_of_bin[rows[side[rows]]] = np.arange(CH_A, NB)

    core_of_bin = np.arange(NBINS) // NB
    slot_of = (core_of_bin[bin_of] * NPAD + pos_of_bin[bin_of] * 128
               + rank_in_bin)
    return slot_of


def host_prep(x, edge_index, batch):
    import ml_dtypes
    src = edge_index[0].astype(np.int64)
    dst = edge_index[1].astype(np.int64)
    deg = np.bincount(dst, minlength=N).astype(np.float64) + 1.0
    dis = (1.0 / np.sqrt(deg)).astype(np.float32)

    if BALANCE:
        slot_of = _balanced_slots(src, dst)
    else:
        slot_of = (np.arange(N, dtype=np.int64) // NPC) * NPAD \
            + (np.arange(N, dtype=np.int64) % NPC)

    sslot = slot_of[src]
    r = sslot // NPAD
    k = sslot % NPAD
    kb = k // 128
    kp = k % 128
    stream = (kb >= CH_A).astype(np.int64)              # 0 = a, 1 = b
    # chunk-major rows: a: r*HA + p*25 + b ; b: r*HB + p*(NB-25) + (b-25)
    loc = np.where(stream == 0,
                   r * HALF_A + kp * CH_A + kb,
                   r * HALF_B + kp * (NB - CH_A) + (kb - CH_A)).astype(np.int64)
    dslot = slot_of[dst]
    core_of = dslot // NPAD
    dloc = dslot % NPAD
    blk = dloc // 128
    id_in_blk = dloc % 128

    key = (core_of * 2 + stream) * NB + blk
    order = np.argsort(key, kind="stable")
    s_loc = loc[order]
    s_id = id_in_blk[order]
    gcnt = np.bincount(key, minlength=C * 2 * NB).reshape(C, 2, NB)
    goff = np.zeros(C * 2 * NB + 1, np.int64)
    np.cumsum(gcnt.reshape(-1), out=goff[1:])

    # shared chunks-per-block (max over cores), per stream
    CPB = [np.maximum.reduce(-(-gcnt[:, s, :] // 128), axis=0) for s in range(2)]
    chunk_blocks = [np.repeat(np.arange(NB), CPB[s]) for s in range(2)]
    S = [int(CPB[s].sum()) * 128 for s in range(2)]
    blk_chunk_start = [np.concatenate([[0], np.cumsum(CPB[s])]) for s in range(2)]

    idx_host = [[None] * C for _ in range(2)]
    ids_host = [[None] * C for _ in range(2)]
    for s in range(2):
        for c in range(C):
            ia = np.zeros(S[s], np.int64)
            da = np.full(S[s], -1.0, np.float32)
            for b in range(NB):
                g = (c * 2 + s) * NB + b
                cnt = goff[g + 1] - goff[g]
                o = int(blk_chunk_start[s][b]) * 128
                ia[o:o + cnt] = s_loc[goff[g]:goff[g + 1]]
                da[o:o + cnt] = s_id[goff[g]:goff[g + 1]]
            idx_host[s][c] = _wrap16(ia)
            ids_host[s][c] = np.ascontiguousarray(
                da.reshape(S[s] // 128, 128).T).astype(ml_dtypes.bfloat16)

    # segment calls: (slot_off, nslots, chunk0, nchunks)
    calls = []
    for s in range(2):
        cl = []
        off = 0
        while off < S[s]:
            n = min(SEG, S[s] - off)
            cl.append((off, n, off // 128, n // 128))
            off += n
        calls.append(cl)

    # per-core aux arrays (scatter node data to its assigned slot)
    sc = slot_of // NPAD
    sl = slot_of % NPAD
    dis_pad = np.zeros((C, NPAD), np.float32)
    dis_pad[sc, sl] = dis
    dis_nm = dis_pad.reshape(C, NB, 128).transpose(0, 2, 1).copy()   # [C,128,NB]
    dis_fm = np.repeat(dis_pad[:, None, :], 128, axis=1)             # [C,128,NPAD]

    x_fm = np.zeros((C, 128, NPAD), np.float32)
    x_fm[sc, :, sl] = np.asarray(x, np.float32)
    if H16:
        x_fm = x_fm.astype(ml_dtypes.bfloat16)

    cnt = np.bincount(batch.astype(np.int64), minlength=NG).astype(np.float64)
    w = (1.0 / np.maximum(cnt, 1.0)).astype(np.float32)
    bl = batch.astype(np.int64)
    pool_mat = np.zeros((C, NPAD, NG), np.float32)
    pool_mat[sc, sl, bl] = w[bl]
    pool_t = pool_mat.reshape(C, NB, 128, NG).transpose(0, 2, 1, 3).reshape(
        C, 128, NB * NG).copy()

    return dict(dis_nm=dis_nm, dis_fm=dis_fm, x_fm=x_fm, pool_t=pool_t,
                idx_host=idx_host, ids_host=ids_host, calls=calls,
                CPB=CPB, chunk_blocks=chunk_blocks, S=S,
                blk_chunk_start=blk_chunk_start)


def build_program(nc, st):
    calls = st["calls"]
    CPB = st["CPB"]
    blk_start = st["blk_chunk_start"]
    S = st["S"]

    # ---- I/O ----
    x_in = nc.dram_tensor("x_fm", [128, NPAD], HDT, kind="ExternalInput")
    w_in = nc.dram_tensor("wcat", [128, NCONV * 128], HDT, kind="ExternalInput")
    b_in = nc.dram_tensor("bcat", [128, NCONV], f32, kind="ExternalInput")
    dnm_in = nc.dram_tensor("dis_nm", [128, NB], f32, kind="ExternalInput")
    dfm_in = nc.dram_tensor("dis_fm", [128, NPAD], f32, kind="ExternalInput")
    pool_in = nc.dram_tensor("pool_t", [128, NB * NG], f32, kind="ExternalInput")
    ident_in = nc.dram_tensor("ident", [128, 128], f32, kind="ExternalInput")
    idx_in = [nc.dram_tensor(f"idx{s}", [128, S[s] // 16], i16,
                             kind="ExternalInput") for s in range(2)]
    ids_in = [nc.dram_tensor(f"ids{s}", [128, S[s] // 128], bf16,
                             kind="ExternalInput") for s in range(2)]
    out_t = nc.dram_tensor("out", [NG, 128], f32, kind="ExternalOutput")
    hdump_t = nc.dram_tensor("hdump", [128, NPAD], f32,
                             kind="ExternalOutput") if DUMP_H else None

    g_slice = [nc.dram_tensor("g_slice0", [HALF_A, 128], bf16, kind="Internal"),
               nc.dram_tensor("g_slice1", [HALF_B, 128], bf16, kind="Internal")]
    # double-buffered by conv parity: AG(i+1) never WAR-waits on conv i's
    # gathers
    g_full = [[nc.dram_tensor(f"g_full{s}_{p}", [NH_A if s == 0 else NH_B, 128],
                              bf16, kind="Internal", addr_space="Shared")
               for p in range(2)] for s in range(2)]
    ar_in = nc.dram_tensor("ar_in", [NG, 128], f32, kind="Internal")
    ar_out = nc.dram_tensor("ar_out", [NG, 128], f32, kind="Internal",
                            addr_space="Shared")
    dum_in = nc.dram_tensor("dum_in", [16, 2], f32, kind="Internal")
    dum_out = nc.dram_tensor("dum_out", [16, 2], f32, kind="Internal",
                             addr_space="Shared")
    rg = [list(range(C))]

    def ap3(t, off_elems, dims):
        return bass.AP(t, off_elems, dims)

    with tile.TileContext(nc) as tc:
        with tc.tile_pool(name="const", bufs=1) as cp, \
             tc.tile_pool(name="state", bufs=1) as sp, \
             tc.tile_pool(name="ph", bufs=2, space="PSUM") as php, \
             tc.tile_pool(name="pagg", bufs=int(os.environ.get("GCN_PAGG", 5)), space="PSUM") as pap:

            b_t = cp.tile([128, NCONV], f32, tag="b")
            dnm_t = cp.tile([128, NB], f32, tag="dnm")
            dfm_t = cp.tile([128, NPAD], f32, tag="dfm")
            ident_t = cp.tile([128, 128], f32, tag="ident")
            ident_bf = cp.tile([128, 128], bf16, tag="identbf")
            iota_seg_f = cp.tile([128, SEG], bf16, tag="iosegf")
            iota_pm_f = cp.tile([128, 128], f32, tag="iopmf")

            idx_res = [cp.tile([128, S[s] // 16], i16, tag=f"idxr{s}",
                               name=f"idxr{s}") for s in range(2)]
            ids_res = [cp.tile([128, S[s] // 128], bf16, tag=f"idsr{s}",
                               name=f"idsr{s}") for s in range(2)]
            h16 = sp.tile([128, NPAD], HDT, tag="h16")
            agg = sp.tile([128, NPAD], f32, tag="agg")
            g_nm = sp.tile([128, NPAD], bf16, tag="gnm")

            nc.sync.dma_start(b_t[:], b_in[:])
            nc.sync.dma_start(dnm_t[:], dnm_in[:])
            nc.sync.dma_start(dfm_t[:], dfm_in[:])
            nc.sync.dma_start(ident_t[:], ident_in[:])
            nc.vector.tensor_copy(ident_bf[:], ident_t[:])
            nc.sync.dma_start(h16[:], x_in[:])
            for s in range(2):
                nc.sync.dma_start(idx_res[s][:], idx_in[s][:])
                nc.sync.dma_start(ids_res[s][:], ids_in[s][:])
            nc.gpsimd.iota(iota_seg_f[:], pattern=[[0, SEGC], [1, 128]],
                           base=0, channel_multiplier=0,
                           allow_small_or_imprecise_dtypes=True)
            nc.gpsimd.iota(iota_pm_f[:], pattern=[[1, 128]], base=0,
                           channel_multiplier=-1,
                           allow_small_or_imprecise_dtypes=True)

            def bs(b):
                return slice(b * 128, (b + 1) * 128)

            def emit_g_dump(col0, col1):
                """Dump g_nm[:, col0:col1) into its slice rows (one stream)."""
                if SKIP_GDMA:
                    return
                half = 0 if col0 < HALF_A else 1
                base = 0 if half == 0 else HALF_A
                hsz = HALF_A if half == 0 else HALF_B
                nc.sync.dma_start(
                    ap3(g_slice[half], col0 - base, [[hsz, 128], [1, col1 - col0]]),
                    g_nm[:, col0:col1])

            def emit_g_dmas(half):
                if half == 0:
                    emit_g_dump(0, HALF_A)
                else:
                    emit_g_dump(HALF_A, NPAD)

            def emit_ag(half, par):
                if SKIP_COLL:
                    return
                nc.gpsimd.collective_compute(
                    "AllGather", AT.bypass, replica_groups=rg,
                    ins=[g_slice[half][:]], outs=[g_full[half][par][:]])

            tailp = tc.alloc_tile_pool(name="tail", bufs=2)
            NBUF = int(os.environ.get("GCN_NBUF", 8))
            mp = tc.alloc_tile_pool(name="msg", bufs=NBUF)
            op = tc.alloc_tile_pool(name="oh", bufs=NBUF)
            tp = tc.alloc_tile_pool(name="meta", bufs=3)

            # per-stream segment state (parameterized by conv parity)
            def make_stream(s, par):
                msg_tiles = {}
                oh_tiles = {}
                state = {"emitted": -1}

                def emit_seg(si):
                    off, n, c0, nch = calls[s][si]
                    msg = mp.tile([128, SEGC, 128], bf16, tag="msg")
                    if not SKIP_GATHER:
                        nc.gpsimd.dma_gather(
                            msg[:, :nch, :], g_full[s][par][:],
                            idx_res[s][:, off // 16:(off + n) // 16],
                            num_idxs=n, num_idxs_reg=n, elem_size=128,
                            single_packet=False,
                            queue_num=si % nc.num_swdge_queues)
                    else:
                        nc.vector.memset(msg[:, :nch, :], 0.0)
                    oh = op.tile([128, SEG], bf16, tag="oh")
                    sl = ids_res[s][:, c0:c0 + nch]
                    in1 = bass.AP(sl.tensor, sl.offset, sl.ap + [[0, 128]])
                    nc.vector.tensor_tensor(
                        oh[:].rearrange("p (c d) -> p c d", d=128)[:, :nch, :],
                        iota_seg_f[:].rearrange("p (c d) -> p c d", d=128)[:, :nch, :],
                        in1, AT.is_equal)
                    msg_tiles[si] = msg
                    oh_tiles[si] = oh

                def prefetch(nseg):
                    while state["emitted"] < nseg - 1:
                        state["emitted"] += 1
                        emit_seg(state["emitted"])

                def get(ch):
                    si, jj = divmod(ch, SEGC)
                    while state["emitted"] < si:
                        state["emitted"] += 1
                        emit_seg(state["emitted"])
                    return (msg_tiles[si][:, jj, :],
                            oh_tiles[si][:, jj * 128:(jj + 1) * 128])

                return get, prefetch

            def emit_gscale(b, ph):
                """g_nm[:, bs(b)] = ph * dis_nm[:, b] (broadcast col)."""
                if DVE_GSCALE:
                    sl = dnm_t[:, b:b + 1]
                    in1 = bass.AP(sl.tensor, sl.offset,
                                  [list(sl.ap[0]), [0, 128]])
                    nc.vector.tensor_tensor(g_nm[:, bs(b)], ph[:], in1,
                                            AT.mult)
                else:
                    nc.scalar.activation(g_nm[:, bs(b)], ph[:], ACTF.Copy,
                                         scale=dnm_t[:, b:b + 1])

            # dump boundaries: halves of each stream-half (block index after
            # whose epilogue the g_nm column range is complete)
            DUMPS = {CH_A // 2: (0, (CH_A // 2 + 1) * 128),
                     CH_A - 1: ((CH_A // 2 + 1) * 128, HALF_A),
                     CH_A + (NB - CH_A) // 2: (HALF_A, (CH_A + (NB - CH_A) // 2 + 1) * 128),
                     NB - 1: ((CH_A + (NB - CH_A) // 2 + 1) * 128, NPAD)}

            # ---- conv 0 warmup: h@W for all blocks, issue AGs ----
            w_t = tp.tile([128, 128], HDT, tag="wt", bufs=2)
            nc.sync.dma_start(w_t[:], w_in[:, 0:128])
            for b in range(NB):
                ph = php.tile([128, 128], f32, tag="ph")
                nc.tensor.matmul(ph[:], h16[:, bs(b)], w_t[:],
                                 start=True, stop=True)
                emit_gscale(b, ph)
                if EARLY_DUMP and b in DUMPS:
                    emit_g_dump(*DUMPS[b])
                if b == CH_A - 1:
                    if not EARLY_DUMP:
                        emit_g_dmas(0)
                    emit_ag(0, 0)
            if not EARLY_DUMP:
                emit_g_dmas(1)
            emit_ag(1, 0)

            pool_tile = None
            ppool = None
            streams = (make_stream(0, 0), make_stream(1, 0))

            for cv in range(NCONV):
                last = (cv == NCONV - 1)
                par = cv % 2
                if last and TAILFOLD:
                    # prefetch pooling matrix; tail runs inside this conv's
                    # b-phase
                    pool_tile = tailp.tile([128, NB * NG], f32, tag="poolm",
                                           bufs=1)
                    nc.sync.dma_start(pool_tile[:], pool_in[:])
                    ppool = pap.tile([NG, 128], f32, tag="ppool", bufs=1)
                for _ in range(DUMMY_COLL):
                    # probe: extra barrier per conv, no data deps
                    nc.gpsimd.collective_compute(
                        "AllReduce", AT.add, replica_groups=rg,
                        ins=[dum_in[:]], outs=[dum_out[:]])
                if AG_NODEP and not last:
                    # timing probe: issue next conv's AGs dependency-free
                    # (reads stale g_slice; results are WRONG)
                    emit_ag(0, (cv + 1) % 2)
                    emit_ag(1, (cv + 1) % 2)
                if not last:
                    w_nt = tp.tile([128, 128], HDT, tag="wt", bufs=2)
                    nc.sync.dma_start(w_nt[:],
                                      w_in[:, (cv + 1) * 128:(cv + 2) * 128])

                (get_a, _), (get_b, _) = streams

                # ---- a-phase: self-loop start + stream-a chunks ----
                for b in range(NB):
                    pa = pap.tile([128, 128], f32, tag="pagg")
                    nch = int(CPB[0][b]) if not SKIP_STREAMS else 0
                    nc.tensor.matmul(pa[:], g_nm[:, bs(b)], ident_bf[:],
                                     start=True, stop=(nch == 0))
                    for j in range(nch):
                        m, o = get_a(int(blk_start[0][b]) + j)
                        nc.tensor.matmul(pa[:], m, o, start=False,
                                         stop=(j == nch - 1))
                    nc.scalar.activation(agg[:, bs(b)], pa[:], ACTF.Copy,
                                         scale=1.0)

                # ---- b-phase: stream-b chunks + fused epilogue +
                #      next conv h@W interleave ----
                for b in range(NB):
                    nch = int(CPB[1][b]) if not SKIP_STREAMS else 0
                    if nch > 0:
                        pb = pap.tile([128, 128], f32, tag="pagg")
                        for j in range(nch):
                            m, o = get_b(int(blk_start[1][b]) + j)
                            nc.tensor.matmul(pb[:], m, o, start=(j == 0),
                                             stop=(j == nch - 1))
                        nc.vector.tensor_tensor(agg[:, bs(b)], pb[:],
                                                agg[:, bs(b)], AT.add)
                    nc.vector.tensor_tensor(agg[:, bs(b)], agg[:, bs(b)],
                                            dfm_t[:, bs(b)], AT.mult)
                    actf = ACTF.Relu if cv % 2 == 0 else ACTF.Identity
                    # last conv: keep final h in f32 (in agg) for the tail
                    ep_dst = agg if last else h16
                    nc.scalar.activation(ep_dst[:, bs(b)], agg[:, bs(b)], actf,
                                         bias=b_t[:, cv:cv + 1], scale=1.0)
                    if last and TAILFOLD:
                        pt = php.tile([128, 128], f32, tag="ph")
                        nc.tensor.transpose(pt[:], agg[:, bs(b)], ident_t[:])
                        hb_t = tailp.tile([128, 128], f32, tag="hnmb")
                        nc.vector.tensor_copy(hb_t[:], pt[:])
                        nc.tensor.matmul(ppool[:],
                                         pool_tile[:, b * NG:(b + 1) * NG],
                                         hb_t[:], start=(b == 0),
                                         stop=(b == NB - 1))
                    if not last:
                        ph = php.tile([128, 128], f32, tag="ph")
                        nc.tensor.matmul(ph[:], h16[:, bs(b)], w_nt[:],
                                         start=True, stop=True)
                        emit_gscale(b, ph)
                        if EARLY_DUMP and b in DUMPS:
                            emit_g_dump(*DUMPS[b])
                        if b == CH_A - 1:
                            if not AG_NODEP:
                                if not EARLY_DUMP:
                                    emit_g_dmas(0)
                                emit_ag(0, (cv + 1) % 2)
                if not last:
                    nstreams = (make_stream(0, (cv + 1) % 2),
                                make_stream(1, (cv + 1) % 2))
                    if not AG_NODEP:
                        if not EARLY_DUMP:
                            emit_g_dmas(1)
                        if PREF_A > 0 and not SKIP_STREAMS:
                            # emit a few next-conv stream-a gathers before the
                            # AG_b trigger so its input-ready wait overlaps
                            # with their descriptor generation
                            nstreams[0][1](min(PREF_A, len(calls[0])))
                        emit_ag(1, (cv + 1) % 2)
                    streams = nstreams

            for p in (tp, op, mp):
                p.release()

            # ---- mean pool + AllReduce ----
            if DUMP_H:
                nc.sync.dma_start(hdump_t[:], agg[:])
            if not TAILFOLD:
                pool_tile = tailp.tile([128, NB * NG], f32, tag="poolm",
                                       bufs=1)
                nc.sync.dma_start(pool_tile[:], pool_in[:])
                hnm = tailp.tile([128, NPAD], f32, tag="hnm", bufs=1)
                for b in range(NB):
                    pt = php.tile([128, 128], f32, tag="ph")
                    nc.tensor.transpose(pt[:], agg[:, bs(b)], ident_t[:])
                    nc.vector.tensor_copy(hnm[:, bs(b)], pt[:])
                ppool = pap.tile([NG, 128], f32, tag="ppool", bufs=1)
                for b in range(NB):
                    nc.tensor.matmul(ppool[:],
                                     pool_tile[:, b * NG:(b + 1) * NG],
                                     hnm[:, bs(b)], start=(b == 0),
                                     stop=(b == NB - 1))
            pres = sp.tile([NG, 128], f32, tag="pres")
            nc.vector.tensor_copy(pres[:], ppool[:])
            nc.sync.dma_start(ar_in[:], pres[:])
            if not SKIP_COLL:
                nc.gpsimd.collective_compute(
                    "AllReduce", AT.add, replica_groups=rg,
                    ins=[ar_in[:]], outs=[ar_out[:]])
            ores = sp.tile([NG, 128], f32, tag="ores")
            nc.sync.dma_start(ores[:], ar_out[:] if not SKIP_COLL else ar_in[:])
            nc.sync.dma_start(out_t[:], ores[:])
            tailp.release()
    return nc


def kernel(x, edge_index, batch, W1, b1, W2, b2, _want_trace=False, _want_res=False):
    x = np.asarray(x)
    edge_index = np.asarray(edge_index)
    batch = np.asarray(batch)
    W1, b1, W2, b2 = (np.asarray(a) for a in (W1, b1, W2, b2))

    st = host_prep(x, edge_index, batch)
    wcat, bcat = build_wcat(W1, b1, W2, b2)

    nc = bacc.Bacc("TRN2", target_bir_lowering=False, debug=False,
                   enable_asserts=False, num_devices=C,
                   num_swdge_queues=int(os.environ.get("GCN_NQ", 3)))
    build_program(nc, st)
    nc.compile()

    ident = np.eye(128, dtype=np.float32)
    in_maps = []
    for c in range(C):
        in_maps.append({
            "x_fm": st["x_fm"][c],
            "wcat": wcat, "bcat": bcat,
            "dis_nm": st["dis_nm"][c], "dis_fm": st["dis_fm"][c],
            "pool_t": st["pool_t"][c], "ident": ident,
            "idx0": st["idx_host"][0][c], "idx1": st["idx_host"][1][c],
            "ids0": st["ids_host"][0][c], "ids1": st["ids_host"][1][c],
        })

    res = run_bass_kernel_spmd(nc, in_maps, core_ids=list(range(C)),
                               trace=_want_trace)
    out = res.results[0]["out"].astype(np.float32)
    if _want_trace or _want_res:
        return out, res
    return out

